# revision 50
# baseline (speedup 1.0000x reference)
"""Local (banded) attention kernel for Trainium2, sharded over 8 NeuronCores.

Sharding: core c handles batch b=c//4 and heads 4*(c%4)..4*(c%4)+3.
Q/K projections and QK^T run as fp8 DoubleRow matmuls (host pre-quantizes
x and the scaled Q/K weight slices, permuted so each head's 64-dim split
lands as [32 partitions x 2 DR slots]).  The band mask is accumulated into
the energy PSUM via tiny fp8e5 DoubleRow matmuls (identity stationary x
precomputed -57344 panels), so exp() needs no separate mask pass.  V and
output projections run in f16; y partials stream out in f16 and the host
sums the 4 partials per batch in f32 and adds the output bias.
"""

import ml_dtypes
import numpy as np

import concourse.bass as bass
import concourse.mybir as mybir
from concourse import bacc
from concourse.tile import TileContext
from concourse.bass_utils import run_bass_kernel_spmd
from concourse.masks import make_identity

B, N, E, H, DH, WIN = 2, 2048, 1024, 16, 64, 128
HPC = 4              # heads per core
SL = HPC * DH        # feature slice per core (256)
NT = N // 128        # 16 query/key tiles
F32 = mybir.dt.float32
F16 = mybir.dt.float16
BF16 = mybir.dt.bfloat16
F8 = mybir.dt.float8e4
F8E5 = mybir.dt.float8e5
SCALE = 1.0 / 32.0   # 1/sqrt(E)
WSCALE = 32.0        # Q/K weights are shipped as fp8(32*W); undone in copies
MASKVAL = -57344.0   # exactly representable in e5m2; /32 => -1792 pre-exp
AUXW = 264           # aux rows: 0=bv, 1=ones, 2=zeros
KO = E // 128        # 8 contraction tiles
KP = KO // 2         # 4 DoubleRow contraction-pair tiles
DR = mybir.MatmulPerfMode.DoubleRow

_CACHED_NC = None


def _build_nc(nobias):
    nc = bacc.Bacc("TRN2", target_bir_lowering=False)

    x8_d = nc.dram_tensor("x8", [E, N], F8, kind="ExternalInput")
    x1_d = nc.dram_tensor("x1", [E, N], F8, kind="ExternalInput")
    wq8_d = nc.dram_tensor("wq8", [E, 2, 128], F8, kind="ExternalInput")
    wk8_d = nc.dram_tensor("wk8", [E, 2, 128], F8, kind="ExternalInput")
    wv0_d = nc.dram_tensor("wv0", [E, SL], F8, kind="ExternalInput")
    wv1_d = nc.dram_tensor("wv1", [E, SL], F8, kind="ExternalInput")
    wv0s_d = nc.dram_tensor("wv0s", [E, SL], F8, kind="ExternalInput")
    wp_d = nc.dram_tensor("wp", [SL, E], F16, kind="ExternalInput")
    bqk_d = nc.dram_tensor("bqk", [128, 6], F32, kind="ExternalInput")
    msk_d = nc.dram_tensor("msk", [128, 2, 256], F8E5, kind="ExternalInput")
    idz_d = nc.dram_tensor("idz", [128, 2, 128], F8E5, kind="ExternalInput")
    y_d = nc.dram_tensor("y", [N, E], F16, kind="ExternalOutput")

    with TileContext(nc) as tc:
        with (
            tc.tile_pool(name="const", bufs=1) as const,
            tc.tile_pool(name="persist", bufs=1) as persist,
            tc.tile_pool(name="io", bufs=3) as io,
            tc.tile_pool(name="small", bufs=6) as small,
            tc.tile_pool(name="att2p", bufs=3) as att2p,
            tc.tile_pool(name="strips", bufs=20) as strip_pool,
            tc.tile_pool(name="ps_mm", bufs=2, space="PSUM") as ps_mm,
            tc.tile_pool(name="ps_e", bufs=2, space="PSUM") as ps_e,
            tc.tile_pool(name="ps_ut", bufs=2, space="PSUM") as ps_ut,
        ):
            # ---- DMAs ordered by first use; the first few go out on the
            # idle Act/DVE queues so their DGE prologues overlap ----
            wq_sb = persist.tile([128, KP, 2, 2, 128], F8)
            wk_sb = persist.tile([128, KP, 2, 2, 128], F8)
            x8_sb = persist.tile([128, KO, N], F8)
            x1_sb = persist.tile([128, KO, N], F8)
            wv0_sb = persist.tile([128, KO, SL], F8)
            wv1_sb = persist.tile([128, KO, SL], F8)
            wv0s_sb = persist.tile([128, KO, SL], F8)
            wp_sb = persist.tile([128, 2, E], F16)
            x8_ap = x8_d.ap().rearrange("(ko p) n -> p ko n", p=128)
            x1_ap = x1_d.ap().rearrange("(ko p) n -> p ko n", p=128)
            NCH = 4
            CW = N // NCH  # 512

            sa, sb = slice(0, 256), slice(256, CW)
            nc.scalar.dma_start(
                wq_sb[:], wq8_d.ap().rearrange(
                    "(kp dr p) i m -> p kp dr i m", p=128, dr=2))
            nc.scalar.dma_start(
                wk_sb[:], wk8_d.ap().rearrange(
                    "(kp dr p) i m -> p kp dr i m", p=128, dr=2))
            bqk = const.tile([128, 6], F32)
            nc.scalar.dma_start(bqk[:], bqk_d.ap())
            msk = const.tile([128, 2, 256], F8E5)
            idz = const.tile([128, 2, 128], F8E5)
            nc.scalar.dma_start(msk[:], msk_d.ap())
            nc.scalar.dma_start(idz[:], idz_d.ap())
            nc.sync.dma_start(x8_sb[:, :, sa], x8_ap[:, :, sa])
            nc.sync.dma_start(x8_sb[:, :, sb], x8_ap[:, :, sb])
            wvr = "(ko p) m -> p ko m"
            nc.sync.dma_start(wv0_sb[:], wv0_d.ap().rearrange(wvr, p=128))
            nc.sync.dma_start(wv1_sb[:], wv1_d.ap().rearrange(wvr, p=128))
            nc.sync.dma_start(wv0s_sb[:], wv0s_d.ap().rearrange(wvr, p=128))
            s0 = slice(0, CW)
            nc.sync.dma_start(x1_sb[:, :, s0], x1_ap[:, :, s0])
            ident = const.tile([128, 128], BF16)
            make_identity(nc, ident[:])
            # warm the PE p-state while input DMAs are in flight: the clock
            # ramps to full after ~3us of continuous matmul activity
            warm = ps_mm.tile([128, 2, 256], F32, tag="mm", name="warm")
            for wi in range(40):
                nc.tensor.matmul(
                    warm[:, 0, :128], lhsT=ident[:], rhs=ident[:],
                    start=(wi == 0), stop=(wi == 39),
                    skip_group_check=True)
            nc.sync.dma_start(
                wp_sb[:], wp_d.ap().rearrange("(g p) f -> p g f", p=128))
            for c4 in range(1, NCH):
                s = slice(c4 * CW, (c4 + 1) * CW)
                nc.sync.dma_start(x8_sb[:, :, s], x8_ap[:, :, s])
                nc.sync.dma_start(x1_sb[:, :, s], x1_ap[:, :, s])

            # ---- projection outputs ----
            # q8/k8 layout (hw only allows AP base partitions 0/32/64):
            #   head 0: partitions  0-31, slots 0/1 = dh halves  (DoubleRow)
            #   head 1: partitions 32-63, slots 0/1 = dh halves  (DoubleRow)
            #   head 2: partitions 64-127, slot 0 = full dh      (plain fp8)
            #   head 3: partitions 64-127, slot 1 = full dh      (plain fp8)
            q8 = persist.tile([128, 2, N], F8, name="q8", tag="q8")
            k8 = persist.tile([128, 2, N], F8, name="k8", tag="k8")
            vaug = persist.tile([128, NT, HPC, DH + 1], BF16)
            nc.gpsimd.memset(vaug[:, :, :, DH], 1.0)

            # ---- phase 2: QKV emitted as drip-able units so projection
            # matmuls (PE) interleave with strip exp/copies (Act/DVE) ----
            def emit_qk_unit(ch, proj, c2):
                w_sb, out_t, bc = ((wq_sb, q8, 0), (wk_sb, k8, 2))[proj]
                cs = slice(ch * CW + c2 * 256, ch * CW + c2 * 256 + 256)
                ps = ps_mm.tile([128, 2, 256], F32, tag="mm", name="ps_qk")
                for i in range(2):
                    for kp in range(KP):
                        nc.tensor.matmul(
                            ps[:, i, :],
                            lhsT=w_sb[:, kp, :, i, :],
                            rhs=x8_sb[:, 2 * kp:2 * kp + 2, cs],
                            start=(kp == 0), stop=(kp == KP - 1),
                            perf_mode=DR)
                if nobias:
                    if proj == 0:
                        nc.scalar.activation(
                            out_t[:, :, cs], ps[:],
                            mybir.ActivationFunctionType.Identity,
                            scale=1.0 / WSCALE)
                    else:
                        nc.vector.tensor_scalar_mul(
                            out_t[:, :, cs], ps[:], 1.0 / WSCALE)
                else:
                    nc.scalar.activation(
                        out_t[:, 0, cs], ps[:, 0, :],
                        mybir.ActivationFunctionType.Identity,
                        scale=1.0 / WSCALE, bias=bqk[:, bc:bc + 1])
                    nc.vector.tensor_scalar(
                        out_t[:, 1, cs], ps[:, 1, :],
                        1.0 / WSCALE, bqk[:, bc + 1:bc + 2],
                        mybir.AluOpType.mult, mybir.AluOpType.add)

            def emit_v_unit(nt):
                # 32*v ~= X0@W0 + X0@(W1/64) + X1@(W0/64); /32 in the copy
                ps = ps_mm.tile([128, 2, 256], F32, tag="mm", name="ps_v")
                psv = ps[:, 0, :]
                rs = slice(nt * 128, (nt + 1) * 128)
                terms = ((x8_sb, wv0_sb), (x8_sb, wv1_sb), (x1_sb, wv0s_sb))
                for ti, (xs, ws) in enumerate(terms):
                    for kp in range(KP):
                        nc.tensor.matmul(
                            psv, lhsT=xs[:, 2 * kp:2 * kp + 2, rs],
                            rhs=ws[:, 2 * kp:2 * kp + 2, :],
                            start=(ti == 0 and kp == 0),
                            stop=(ti == 2 and kp == KP - 1),
                            perf_mode=DR)
                nc.vector.tensor_scalar_mul(
                    vaug[:, nt, :, :DH],
                    psv.rearrange("p (h d) -> p h d", d=DH), 1.0 / WSCALE)

            def chunk_units(ch, qk_only=False, v_only=False):
                # (sort_key, deadline_kj, unit): qk needed by strips at
                # kj=4ch-1, v unit nt by process_tile(nt-1) at kj=nt+1.
                # sort keys stagger v after qk (v DMAs arrive later).
                us = []
                if not v_only:
                    us += [(4 * ch - 1, 4 * ch - 1,
                            lambda p=p, c=c: emit_qk_unit(ch, p, c))
                           for p in range(2) for c in range(2)]
                if not qk_only:
                    us += [(n + 1, n + 1, lambda n=n: emit_v_unit(n))
                           for n in range(ch * NCH, (ch + 1) * NCH)]
                return us

            # ---- phase 3+4: banded attention, fused projection + store ----
            strips = {}

            def emit_strip_pair(hp, kj):
                """Strips for heads (2*hp, 2*hp+1) of key tile kj: QK^T +
                mask matmuls into a 2-bank psum tile, one fused exp."""
                lo, hi = max(0, kj - 1), min(NT - 1, kj + 1)
                w = (hi - lo + 1) * 128
                kjs = slice(kj * 128, (kj + 1) * 128)
                sps = slice(lo * 128, (hi + 1) * 128)
                panels = []
                if kj > 0:   # U panel sits where queries tile kj-1 lives
                    panels.append(((kj - 1 - lo) * 128, 0))
                if kj < NT - 1:  # L panel at queries tile kj+1
                    panels.append(((kj + 1 - lo) * 128, 128))
                pe2 = ps_e.tile([128, 2, 512], F32, tag="pe", name="pe2")
                for j in range(2):
                    h = 2 * hp + j
                    if h < 2:
                        hs = slice(32 * h, 32 * h + 32)
                        nc.tensor.matmul(
                            pe2[:, j, :w], lhsT=k8[hs, :, kjs],
                            rhs=q8[hs, :, sps],
                            start=True, stop=False, perf_mode=DR,
                            skip_group_check=True)
                    else:
                        i = h - 2
                        nc.tensor.matmul(
                            pe2[:, j, :w], lhsT=k8[64:128, i, kjs],
                            rhs=q8[64:128, i, sps],
                            start=True, stop=False, skip_group_check=True)
                    for n_, (po, mo) in enumerate(panels):
                        nc.tensor.matmul(
                            pe2[:, j, po:po + 128],
                            lhsT=idz[:],
                            rhs=msk[:, :, mo:mo + 128],
                            start=False, stop=(n_ == len(panels) - 1),
                            perf_mode=DR, skip_group_check=True)
                st2 = strip_pool.tile(
                    [128, 2, 384], BF16, tag="strip", name="st2")
                nc.scalar.activation(
                    st2[:, :, :w], pe2[:, :, :w],
                    mybir.ActivationFunctionType.Exp, scale=SCALE)
                strips[(2 * hp, kj)] = (st2[:, 0, :], lo)
                strips[(2 * hp + 1, kj)] = (st2[:, 1, :], lo)

            tile_ao = {}

            def process_tile_a(t):
                ks = [k for k in (t - 1, t, t + 1) if 0 <= k < NT]
                pu = ps_ut.tile([128, HPC, DH + 1], F32, tag="ut", name="pu")
                # PE executes in emission order, so the first matmul's
                # start=True zeroes the whole 2KB bank region before the
                # other 11 accumulate into their sub-slices.
                for h in range(HPC):
                    for i, k2 in enumerate(ks):
                        st, lo2 = strips[(h, k2)]
                        col = (t - lo2) * 128
                        nc.tensor.matmul(
                            pu[:, h, :], lhsT=st[:, col:col + 128],
                            rhs=vaug[:, k2, h, :],
                            start=(h == 0 and i == 0),
                            stop=(h == HPC - 1 and i == len(ks) - 1),
                            skip_group_check=True)
                rec = small.tile([128, HPC], F32, tag="rec", name="rec")
                nc.vector.reciprocal(rec[:], pu[:, :, DH])
                ao = small.tile([128, HPC, DH], BF16, tag="ao", name="ao")
                nc.vector.tensor_tensor(
                    ao[:], pu[:, :, :DH],
                    rec[:].unsqueeze(2).broadcast_to([128, HPC, DH]),
                    mybir.AluOpType.mult)
                tile_ao[t] = ao

            def process_tile_b(t):
                ts_ = slice(t * 128, (t + 1) * 128)
                ao = tile_ao.pop(t)
                ptb = ps_ut.tile([128, 2, 128], BF16, tag="ut", name="ptb")
                for g in range(2):
                    nc.tensor.transpose(
                        ptb[:, g, :], ao[:, 2 * g:2 * g + 2, :], ident[:])
                att2 = att2p.tile([128, 2, 128], BF16, tag="att2", name="att2")
                if nobias:
                    nc.vector.tensor_copy(
                        att2.rearrange("p g n -> p (g n)"),
                        ptb.rearrange("p g n -> p (g n)"))
                else:
                    for g in range(2):
                        nc.vector.tensor_scalar_add(
                            att2[:, g, :], ptb[:, g, :], bqk[:, 4 + g:5 + g])
                # fused output projection for this token tile
                y_sb = io.tile([128, E], F16, tag="y", name="y_sb")
                for fc in range(2):
                    ps = ps_mm.tile([128, 2, 256], F32, tag="mm", name="ps_y")
                    psy = ps.rearrange("p a b -> p (a b)")
                    fs = slice(fc * 512, (fc + 1) * 512)
                    for g in range(2):
                        nc.tensor.matmul(
                            psy,
                            lhsT=att2[:, g, :],
                            rhs=wp_sb[:, g, fs],
                            start=(g == 0), stop=(g == 1))
                    if fc == 0 and t % 2 == 0:
                        nc.scalar.activation(
                            y_sb[:, fs], psy,
                            mybir.ActivationFunctionType.Copy)
                    else:
                        nc.vector.tensor_copy(y_sb[:, fs], psy)
                nc.sync.dma_start(y_d[ts_, :], y_sb[:])

            # interleave: chunk 0's qkv first, then per-kj strips/tile with
            # 2-3 qkv units of upcoming chunks dripped in between.  strip kj
            # needs q/k tokens up to 128*(kj+2) => all chunks < (kj+3)/4.
            LEAD = 3
            pending = []  # (sort_key, deadline_kj, unit)
            for ch in range(1, NCH):
                pending.extend(chunk_units(ch))
            pending.sort(key=lambda du: du[0])
            for _, _, u in chunk_units(0):
                u()
            for kj in range(NT):
                while pending and min(d for _, d, _ in pending) <= kj:
                    i = next(i for i, (_, d, _) in enumerate(pending)
                             if d <= kj)
                    pending.pop(i)[2]()
                for hp in range(2):
                    emit_strip_pair(hp, kj)
                if kj >= LEAD:
                    process_tile_a(kj - LEAD)
                if kj >= LEAD + 1:
                    process_tile_b(kj - LEAD - 1)
                # steady drip keeps PE fed between strips through the tail
                drip = 1 if len(pending) <= (NT - 1 - kj) else 2
                for _ in range(min(drip, len(pending))):
                    pending.pop(0)[2]()
            # drain: interleave AV phases with output phases so PE's
            # in-order queue alternates between independent tiles
            process_tile_a(NT - 3)
            process_tile_b(NT - 4)
            process_tile_a(NT - 2)
            process_tile_b(NT - 3)
            process_tile_a(NT - 1)
            process_tile_b(NT - 2)
            process_tile_b(NT - 1)

    nc.compile()
    return nc


_CACHED_NOBIAS = None


def _get_nc(nobias):
    global _CACHED_NC, _CACHED_NOBIAS
    if _CACHED_NC is None or _CACHED_NOBIAS != nobias:
        _CACHED_NC = _build_nc(nobias)
        _CACHED_NOBIAS = nobias
    return _CACHED_NC


def _prep_core(x_b, Wq, bq, Wk, bk, Wv, bv, Wp, gq):
    f8 = ml_dtypes.float8_e4m3
    f8e5 = ml_dtypes.float8_e5m2
    sl = slice(SL * gq, SL * (gq + 1))
    xT = np.ascontiguousarray(x_b.T).astype(np.float32)

    # feature index (within this core's 256-slice) at (slot i, partition m):
    #   m<32: head0 dh=32i+m; 32<=m<64: head1 dh=32i+(m-32);
    #   m>=64: head (2+i), dh=m-64
    fidx = np.zeros((2, 128), np.int64)
    m = np.arange(128)
    for i in range(2):
        fidx[i, :32] = 32 * i + m[:32]
        fidx[i, 32:64] = 64 + 32 * i + (m[32:64] - 32)
        fidx[i, 64:] = 64 * (2 + i) + (m[64:] - 64)

    def qk_weight(W):
        w = np.ascontiguousarray(W[sl].T).astype(np.float32) * WSCALE
        return w[:, fidx].astype(f8)  # [E, 2, 128]

    def qk_bias(b):
        return np.asarray(b, np.float32)[sl][fidx]  # [2, 128]

    bq2, bk2 = qk_bias(bq), qk_bias(bk)
    bvs = np.asarray(bv, np.float32)[sl]
    bqk = np.stack([bq2[0], bq2[1], bk2[0], bk2[1],
                    bvs[:128], bvs[128:]], axis=1)  # [128, 6]

    # band-mask panels: U keeps qcol >= p, L keeps qcol <= p
    pi = np.arange(128)
    msk = np.zeros((128, 2, 256), np.float32)
    msk[:, 0, :128] = np.where(pi[None, :] >= pi[:, None], 0.0, MASKVAL)
    msk[:, 0, 128:] = np.where(pi[None, :] <= pi[:, None], 0.0, MASKVAL)
    idz = np.zeros((128, 2, 128), np.float32)
    idz[:, 0, :] = np.eye(128, dtype=np.float32)

    x0 = xT.astype(f8)
    x1 = ((xT - x0.astype(np.float32)) * 64.0).astype(f8)
    wvT = np.ascontiguousarray(np.asarray(Wv, np.float32)[sl].T)
    wv0 = (WSCALE * wvT).astype(f8)
    wv1 = ((WSCALE * wvT - wv0.astype(np.float32)) * 64.0).astype(f8)
    wv1 = (wv1.astype(np.float32) / 64.0).astype(f8)   # ~= rw, exact-ish
    wv0s = (wv0.astype(np.float32) / 64.0).astype(f8)  # = W0/64, exact-ish
    return {
        "x8": x0,
        "x1": x1,
        "wq8": qk_weight(Wq),
        "wk8": qk_weight(Wk),
        "wv0": wv0,
        "wv1": wv1,
        "wv0s": wv0s,
        "wp": np.ascontiguousarray(np.asarray(Wp, np.float32)[:, sl].T).astype(
            np.float16),
        "bqk": np.ascontiguousarray(bqk),
        "msk": msk.astype(f8e5),
        "idz": idz.astype(f8e5),
    }


def kernel(x, Wq, bq, Wk, bk, Wv, bv, Wp, bp):
    nobias = bool(
        not np.any(np.asarray(bq)) and not np.any(np.asarray(bk))
        and not np.any(np.asarray(bv)))
    nc = _get_nc(nobias)
    x = np.asarray(x, np.float32)
    in_maps = []
    for c in range(8):
        b, gq = c // 4, c % 4
        m = _prep_core(x[b], np.asarray(Wq, np.float32), bq,
                       np.asarray(Wk, np.float32), bk,
                       np.asarray(Wv, np.float32), bv,
                       np.asarray(Wp, np.float32), gq)
        in_maps.append(m)
    res = run_bass_kernel_spmd(nc, in_maps, core_ids=list(range(8)))
    ys = [res.results[c]["y"].astype(np.float32) for c in range(8)]
    bp = np.asarray(bp, np.float32)
    y = np.stack([
        ys[0] + ys[1] + ys[2] + ys[3],
        ys[4] + ys[5] + ys[6] + ys[7],
    ]) + bp[None, None, :]
    return y.astype(np.float32)


# revision 51
# speedup vs baseline: 1.0087x; 1.0087x over previous
"""Local (banded) attention kernel for Trainium2, sharded over 8 NeuronCores.

Sharding: core c handles batch b=c//4 and heads 4*(c%4)..4*(c%4)+3.
Q/K projections and QK^T run as fp8 DoubleRow matmuls (host pre-quantizes
x and the scaled Q/K weight slices, permuted so each head's 64-dim split
lands as [32 partitions x 2 DR slots]).  The band mask is accumulated into
the energy PSUM via tiny fp8e5 DoubleRow matmuls (identity stationary x
precomputed -57344 panels), so exp() needs no separate mask pass.  V and
output projections run in f16; y partials stream out in f16 and the host
sums the 4 partials per batch in f32 and adds the output bias.
"""

import ml_dtypes
import numpy as np

import concourse.bass as bass
import concourse.mybir as mybir
from concourse import bacc
from concourse.tile import TileContext
from concourse.bass_utils import run_bass_kernel_spmd
from concourse.masks import make_identity

B, N, E, H, DH, WIN = 2, 2048, 1024, 16, 64, 128
HPC = 4              # heads per core
SL = HPC * DH        # feature slice per core (256)
NT = N // 128        # 16 query/key tiles
F32 = mybir.dt.float32
F16 = mybir.dt.float16
BF16 = mybir.dt.bfloat16
F8 = mybir.dt.float8e4
F8E5 = mybir.dt.float8e5
SCALE = 1.0 / 32.0   # 1/sqrt(E)
WSCALE = 32.0        # Q/K weights are shipped as fp8(32*W); undone in copies
MASKVAL = -57344.0   # exactly representable in e5m2; /32 => -1792 pre-exp
AUXW = 264           # aux rows: 0=bv, 1=ones, 2=zeros
KO = E // 128        # 8 contraction tiles
KP = KO // 2         # 4 DoubleRow contraction-pair tiles
DR = mybir.MatmulPerfMode.DoubleRow

_CACHED_NC = None


def _build_nc(nobias):
    nc = bacc.Bacc("TRN2", target_bir_lowering=False)

    x8_d = nc.dram_tensor("x8", [E, N], F8, kind="ExternalInput")
    x1_d = nc.dram_tensor("x1", [E, N], F8, kind="ExternalInput")
    wq8_d = nc.dram_tensor("wq8", [E, 2, 128], F8, kind="ExternalInput")
    wk8_d = nc.dram_tensor("wk8", [E, 2, 128], F8, kind="ExternalInput")
    wv0_d = nc.dram_tensor("wv0", [E, SL], F8, kind="ExternalInput")
    wv1_d = nc.dram_tensor("wv1", [E, SL], F8, kind="ExternalInput")
    wv0s_d = nc.dram_tensor("wv0s", [E, SL], F8, kind="ExternalInput")
    wp_d = nc.dram_tensor("wp", [SL, E], F16, kind="ExternalInput")
    bqk_d = nc.dram_tensor("bqk", [128, 6], F32, kind="ExternalInput")
    msk_d = nc.dram_tensor("msk", [128, 2, 256], F8E5, kind="ExternalInput")
    idz_d = nc.dram_tensor("idz", [128, 2, 128], F8E5, kind="ExternalInput")
    y_d = nc.dram_tensor("y", [N, E], F16, kind="ExternalOutput")

    with TileContext(nc) as tc:
        with (
            tc.tile_pool(name="const", bufs=1) as const,
            tc.tile_pool(name="persist", bufs=1) as persist,
            tc.tile_pool(name="io", bufs=3) as io,
            tc.tile_pool(name="small", bufs=6) as small,
            tc.tile_pool(name="att2p", bufs=3) as att2p,
            tc.tile_pool(name="strips", bufs=20) as strip_pool,
            tc.tile_pool(name="ps_mm", bufs=2, space="PSUM") as ps_mm,
            tc.tile_pool(name="ps_e", bufs=2, space="PSUM") as ps_e,
            tc.tile_pool(name="ps_ut", bufs=2, space="PSUM") as ps_ut,
        ):
            # ---- DMAs ordered by first use; the first few go out on the
            # idle Act/DVE queues so their DGE prologues overlap ----
            wq_sb = persist.tile([128, KP, 2, 2, 128], F8)
            wk_sb = persist.tile([128, KP, 2, 2, 128], F8)
            x8_sb = persist.tile([128, KO, N], F8)
            x1_sb = persist.tile([128, KO, N], F8)
            wv0_sb = persist.tile([128, KO, SL], F8)
            wv1_sb = persist.tile([128, KO, SL], F8)
            wv0s_sb = persist.tile([128, KO, SL], F8)
            wp_sb = persist.tile([128, 2, E], F16)
            x8_ap = x8_d.ap().rearrange("(ko p) n -> p ko n", p=128)
            x1_ap = x1_d.ap().rearrange("(ko p) n -> p ko n", p=128)
            NCH = 4
            CW = N // NCH  # 512

            sa, sb = slice(0, 256), slice(256, CW)
            nc.scalar.dma_start(
                wq_sb[:], wq8_d.ap().rearrange(
                    "(kp dr p) i m -> p kp dr i m", p=128, dr=2))
            nc.scalar.dma_start(
                wk_sb[:], wk8_d.ap().rearrange(
                    "(kp dr p) i m -> p kp dr i m", p=128, dr=2))
            bqk = const.tile([128, 6], F32)
            nc.scalar.dma_start(bqk[:], bqk_d.ap())
            msk = const.tile([128, 2, 256], F8E5)
            idz = const.tile([128, 2, 128], F8E5)
            nc.scalar.dma_start(msk[:], msk_d.ap())
            nc.scalar.dma_start(idz[:], idz_d.ap())
            nc.sync.dma_start(x8_sb[:, :, sa], x8_ap[:, :, sa])
            nc.sync.dma_start(x8_sb[:, :, sb], x8_ap[:, :, sb])
            wvr = "(ko p) m -> p ko m"
            nc.sync.dma_start(wv0_sb[:], wv0_d.ap().rearrange(wvr, p=128))
            nc.sync.dma_start(wv1_sb[:], wv1_d.ap().rearrange(wvr, p=128))
            nc.sync.dma_start(wv0s_sb[:], wv0s_d.ap().rearrange(wvr, p=128))
            s0 = slice(0, CW)
            nc.sync.dma_start(x1_sb[:, :, s0], x1_ap[:, :, s0])
            ident = const.tile([128, 128], BF16)
            make_identity(nc, ident[:])
            # warm the PE p-state while input DMAs are in flight: the clock
            # ramps to full after ~3us of continuous matmul activity
            warm = ps_mm.tile([128, 2, 256], F32, tag="mm", name="warm")
            for wi in range(40):
                nc.tensor.matmul(
                    warm[:, 0, :128], lhsT=ident[:], rhs=ident[:],
                    start=(wi == 0), stop=(wi == 39),
                    skip_group_check=True)
            nc.sync.dma_start(
                wp_sb[:], wp_d.ap().rearrange("(g p) f -> p g f", p=128))
            for c4 in range(1, NCH):
                s = slice(c4 * CW, (c4 + 1) * CW)
                nc.sync.dma_start(x8_sb[:, :, s], x8_ap[:, :, s])
                nc.sync.dma_start(x1_sb[:, :, s], x1_ap[:, :, s])

            # ---- projection outputs ----
            # q8/k8 layout (hw only allows AP base partitions 0/32/64):
            #   head 0: partitions  0-31, slots 0/1 = dh halves  (DoubleRow)
            #   head 1: partitions 32-63, slots 0/1 = dh halves  (DoubleRow)
            #   head 2: partitions 64-127, slot 0 = full dh      (plain fp8)
            #   head 3: partitions 64-127, slot 1 = full dh      (plain fp8)
            q8 = persist.tile([128, 2, N], F8, name="q8", tag="q8")
            k8 = persist.tile([128, 2, N], F8, name="k8", tag="k8")
            vaug = persist.tile([128, NT, HPC, DH + 1], BF16)
            nc.gpsimd.memset(vaug[:, :, :, DH], 1.0)

            # ---- phase 2: QKV emitted as drip-able units so projection
            # matmuls (PE) interleave with strip exp/copies (Act/DVE) ----
            def emit_qk_unit(ch, proj, c2):
                w_sb, out_t, bc = ((wq_sb, q8, 0), (wk_sb, k8, 2))[proj]
                cs = slice(ch * CW + c2 * 256, ch * CW + c2 * 256 + 256)
                ps = ps_mm.tile([128, 2, 256], F32, tag="mm", name="ps_qk")
                for i in range(2):
                    for kp in range(KP):
                        nc.tensor.matmul(
                            ps[:, i, :],
                            lhsT=w_sb[:, kp, :, i, :],
                            rhs=x8_sb[:, 2 * kp:2 * kp + 2, cs],
                            start=(kp == 0), stop=(kp == KP - 1),
                            perf_mode=DR)
                if nobias:
                    if proj == 0:
                        nc.scalar.activation(
                            out_t[:, :, cs], ps[:],
                            mybir.ActivationFunctionType.Identity,
                            scale=1.0 / WSCALE)
                    else:
                        nc.vector.tensor_scalar_mul(
                            out_t[:, :, cs], ps[:], 1.0 / WSCALE)
                else:
                    nc.scalar.activation(
                        out_t[:, 0, cs], ps[:, 0, :],
                        mybir.ActivationFunctionType.Identity,
                        scale=1.0 / WSCALE, bias=bqk[:, bc:bc + 1])
                    nc.vector.tensor_scalar(
                        out_t[:, 1, cs], ps[:, 1, :],
                        1.0 / WSCALE, bqk[:, bc + 1:bc + 2],
                        mybir.AluOpType.mult, mybir.AluOpType.add)

            def emit_v_unit(nt):
                # 32*v ~= X0@W0 + X0@(W1/64) + X1@(W0/64); /32 in the copy
                ps = ps_mm.tile([128, 2, 256], F32, tag="mm", name="ps_v")
                psv = ps[:, 0, :]
                rs = slice(nt * 128, (nt + 1) * 128)
                terms = ((x8_sb, wv0_sb), (x8_sb, wv1_sb), (x1_sb, wv0s_sb))
                for ti, (xs, ws) in enumerate(terms):
                    for kp in range(KP):
                        nc.tensor.matmul(
                            psv, lhsT=xs[:, 2 * kp:2 * kp + 2, rs],
                            rhs=ws[:, 2 * kp:2 * kp + 2, :],
                            start=(ti == 0 and kp == 0),
                            stop=(ti == 2 and kp == KP - 1),
                            perf_mode=DR)
                nc.vector.tensor_scalar_mul(
                    vaug[:, nt, :, :DH],
                    psv.rearrange("p (h d) -> p h d", d=DH), 1.0 / WSCALE)

            def chunk_units(ch, qk_only=False, v_only=False):
                # (sort_key, deadline_kj, unit): qk needed by strips at
                # kj=4ch-1, v unit nt by process_tile(nt-1) at kj=nt+1.
                # sort keys stagger v after qk (v DMAs arrive later).
                us = []
                if not v_only:
                    us += [(4 * ch - 1, 4 * ch - 1,
                            lambda p=p, c=c: emit_qk_unit(ch, p, c))
                           for p in range(2) for c in range(2)]
                if not qk_only:
                    us += [(n + 1, n + 1, lambda n=n: emit_v_unit(n))
                           for n in range(ch * NCH, (ch + 1) * NCH)]
                return us

            # ---- phase 3+4: banded attention, fused projection + store ----
            strips = {}

            def emit_strip_pair(hp, kj):
                """Strips for heads (2*hp, 2*hp+1) of key tile kj: QK^T +
                mask matmuls into a 2-bank psum tile, one fused exp."""
                lo, hi = max(0, kj - 1), min(NT - 1, kj + 1)
                w = (hi - lo + 1) * 128
                kjs = slice(kj * 128, (kj + 1) * 128)
                sps = slice(lo * 128, (hi + 1) * 128)
                panels = []
                if kj > 0:   # U panel sits where queries tile kj-1 lives
                    panels.append(((kj - 1 - lo) * 128, 0))
                if kj < NT - 1:  # L panel at queries tile kj+1
                    panels.append(((kj + 1 - lo) * 128, 128))
                pe2 = ps_e.tile([128, 2, 512], F32, tag="pe", name="pe2")
                for j in range(2):
                    h = 2 * hp + j
                    if h < 2:
                        hs = slice(32 * h, 32 * h + 32)
                        nc.tensor.matmul(
                            pe2[:, j, :w], lhsT=k8[hs, :, kjs],
                            rhs=q8[hs, :, sps],
                            start=True, stop=False, perf_mode=DR,
                            skip_group_check=True)
                    else:
                        i = h - 2
                        nc.tensor.matmul(
                            pe2[:, j, :w], lhsT=k8[64:128, i, kjs],
                            rhs=q8[64:128, i, sps],
                            start=True, stop=False, skip_group_check=True)
                    for n_, (po, mo) in enumerate(panels):
                        nc.tensor.matmul(
                            pe2[:, j, po:po + 128],
                            lhsT=idz[:],
                            rhs=msk[:, :, mo:mo + 128],
                            start=False, stop=(n_ == len(panels) - 1),
                            perf_mode=DR, skip_group_check=True)
                st2 = strip_pool.tile(
                    [128, 2, 384], BF16, tag="strip", name="st2")
                nc.scalar.activation(
                    st2[:, :, :w], pe2[:, :, :w],
                    mybir.ActivationFunctionType.Exp, scale=SCALE)
                strips[(2 * hp, kj)] = (st2[:, 0, :], lo)
                strips[(2 * hp + 1, kj)] = (st2[:, 1, :], lo)

            tile_ao = {}

            def process_tile_a(t):
                ks = [k for k in (t - 1, t, t + 1) if 0 <= k < NT]
                pu = ps_ut.tile([128, HPC, DH + 1], F32, tag="ut", name="pu")
                # PE executes in emission order, so the first matmul's
                # start=True zeroes the whole 2KB bank region before the
                # other 11 accumulate into their sub-slices.
                for h in range(HPC):
                    for i, k2 in enumerate(ks):
                        st, lo2 = strips[(h, k2)]
                        col = (t - lo2) * 128
                        nc.tensor.matmul(
                            pu[:, h, :], lhsT=st[:, col:col + 128],
                            rhs=vaug[:, k2, h, :],
                            start=(h == 0 and i == 0),
                            stop=(h == HPC - 1 and i == len(ks) - 1),
                            skip_group_check=True)
                rec = small.tile([128, HPC], F32, tag="rec", name="rec")
                nc.vector.reciprocal(rec[:], pu[:, :, DH])
                ao = small.tile([128, HPC, DH], BF16, tag="ao", name="ao")
                nc.vector.tensor_tensor(
                    ao[:], pu[:, :, :DH],
                    rec[:].unsqueeze(2).broadcast_to([128, HPC, DH]),
                    mybir.AluOpType.mult)
                tile_ao[t] = ao

            def process_tile_b(t):
                ts_ = slice(t * 128, (t + 1) * 128)
                ao = tile_ao.pop(t)
                ptb = ps_ut.tile([128, 2, 128], BF16, tag="ut", name="ptb")
                for g in range(2):
                    nc.tensor.transpose(
                        ptb[:, g, :], ao[:, 2 * g:2 * g + 2, :], ident[:])
                att2 = att2p.tile([128, 2, 128], BF16, tag="att2", name="att2")
                if nobias:
                    nc.vector.tensor_copy(
                        att2.rearrange("p g n -> p (g n)"),
                        ptb.rearrange("p g n -> p (g n)"))
                else:
                    for g in range(2):
                        nc.vector.tensor_scalar_add(
                            att2[:, g, :], ptb[:, g, :], bqk[:, 4 + g:5 + g])
                # fused output projection for this token tile
                y_sb = io.tile([128, E], F16, tag="y", name="y_sb")
                for fc in range(2):
                    ps = ps_mm.tile([128, 2, 256], F32, tag="mm", name="ps_y")
                    psy = ps.rearrange("p a b -> p (a b)")
                    fs = slice(fc * 512, (fc + 1) * 512)
                    for g in range(2):
                        nc.tensor.matmul(
                            psy,
                            lhsT=att2[:, g, :],
                            rhs=wp_sb[:, g, fs],
                            start=(g == 0), stop=(g == 1))
                    if fc == 0 and t % 2 == 0:
                        nc.scalar.activation(
                            y_sb[:, fs], psy,
                            mybir.ActivationFunctionType.Copy)
                    else:
                        nc.vector.tensor_copy(y_sb[:, fs], psy)
                nc.sync.dma_start(y_d[ts_, :], y_sb[:])

            # interleave: chunk 0's qkv first, then per-kj strips/tile with
            # 2-3 qkv units of upcoming chunks dripped in between.  strip kj
            # needs q/k tokens up to 128*(kj+2) => all chunks < (kj+3)/4.
            LEAD = 3
            pending = []  # (sort_key, deadline_kj, unit)
            for ch in range(1, NCH):
                pending.extend(chunk_units(ch))
            pending.sort(key=lambda du: du[0])
            for _, _, u in chunk_units(0):
                u()
            for kj in range(NT):
                while pending and min(d for _, d, _ in pending) <= kj:
                    i = next(i for i, (_, d, _) in enumerate(pending)
                             if d <= kj)
                    pending.pop(i)[2]()
                for hp in range(2):
                    emit_strip_pair(hp, kj)
                if kj >= LEAD:
                    process_tile_a(kj - LEAD)
                    process_tile_b(kj - LEAD)
                # steady drip keeps PE fed between strips through the tail
                drip = 1 if len(pending) <= (NT - 1 - kj) else 2
                for _ in range(min(drip, len(pending))):
                    pending.pop(0)[2]()
            for t in range(NT - LEAD, NT):
                process_tile_a(t)
                process_tile_b(t)

    nc.compile()
    return nc


_CACHED_NOBIAS = None


def _get_nc(nobias):
    global _CACHED_NC, _CACHED_NOBIAS
    if _CACHED_NC is None or _CACHED_NOBIAS != nobias:
        _CACHED_NC = _build_nc(nobias)
        _CACHED_NOBIAS = nobias
    return _CACHED_NC


def _prep_core(x_b, Wq, bq, Wk, bk, Wv, bv, Wp, gq):
    f8 = ml_dtypes.float8_e4m3
    f8e5 = ml_dtypes.float8_e5m2
    sl = slice(SL * gq, SL * (gq + 1))
    xT = np.ascontiguousarray(x_b.T).astype(np.float32)

    # feature index (within this core's 256-slice) at (slot i, partition m):
    #   m<32: head0 dh=32i+m; 32<=m<64: head1 dh=32i+(m-32);
    #   m>=64: head (2+i), dh=m-64
    fidx = np.zeros((2, 128), np.int64)
    m = np.arange(128)
    for i in range(2):
        fidx[i, :32] = 32 * i + m[:32]
        fidx[i, 32:64] = 64 + 32 * i + (m[32:64] - 32)
        fidx[i, 64:] = 64 * (2 + i) + (m[64:] - 64)

    def qk_weight(W):
        w = np.ascontiguousarray(W[sl].T).astype(np.float32) * WSCALE
        return w[:, fidx].astype(f8)  # [E, 2, 128]

    def qk_bias(b):
        return np.asarray(b, np.float32)[sl][fidx]  # [2, 128]

    bq2, bk2 = qk_bias(bq), qk_bias(bk)
    bvs = np.asarray(bv, np.float32)[sl]
    bqk = np.stack([bq2[0], bq2[1], bk2[0], bk2[1],
                    bvs[:128], bvs[128:]], axis=1)  # [128, 6]

    # band-mask panels: U keeps qcol >= p, L keeps qcol <= p
    pi = np.arange(128)
    msk = np.zeros((128, 2, 256), np.float32)
    msk[:, 0, :128] = np.where(pi[None, :] >= pi[:, None], 0.0, MASKVAL)
    msk[:, 0, 128:] = np.where(pi[None, :] <= pi[:, None], 0.0, MASKVAL)
    idz = np.zeros((128, 2, 128), np.float32)
    idz[:, 0, :] = np.eye(128, dtype=np.float32)

    x0 = xT.astype(f8)
    x1 = ((xT - x0.astype(np.float32)) * 64.0).astype(f8)
    wvT = np.ascontiguousarray(np.asarray(Wv, np.float32)[sl].T)
    wv0 = (WSCALE * wvT).astype(f8)
    wv1 = ((WSCALE * wvT - wv0.astype(np.float32)) * 64.0).astype(f8)
    wv1 = (wv1.astype(np.float32) / 64.0).astype(f8)   # ~= rw, exact-ish
    wv0s = (wv0.astype(np.float32) / 64.0).astype(f8)  # = W0/64, exact-ish
    return {
        "x8": x0,
        "x1": x1,
        "wq8": qk_weight(Wq),
        "wk8": qk_weight(Wk),
        "wv0": wv0,
        "wv1": wv1,
        "wv0s": wv0s,
        "wp": np.ascontiguousarray(np.asarray(Wp, np.float32)[:, sl].T).astype(
            np.float16),
        "bqk": np.ascontiguousarray(bqk),
        "msk": msk.astype(f8e5),
        "idz": idz.astype(f8e5),
    }


def kernel(x, Wq, bq, Wk, bk, Wv, bv, Wp, bp):
    nobias = bool(
        not np.any(np.asarray(bq)) and not np.any(np.asarray(bk))
        and not np.any(np.asarray(bv)))
    nc = _get_nc(nobias)
    x = np.asarray(x, np.float32)
    in_maps = []
    for c in range(8):
        b, gq = c // 4, c % 4
        m = _prep_core(x[b], np.asarray(Wq, np.float32), bq,
                       np.asarray(Wk, np.float32), bk,
                       np.asarray(Wv, np.float32), bv,
                       np.asarray(Wp, np.float32), gq)
        in_maps.append(m)
    res = run_bass_kernel_spmd(nc, in_maps, core_ids=list(range(8)))
    ys = [res.results[c]["y"].astype(np.float32) for c in range(8)]
    bp = np.asarray(bp, np.float32)
    y = np.stack([
        ys[0] + ys[1] + ys[2] + ys[3],
        ys[4] + ys[5] + ys[6] + ys[7],
    ]) + bp[None, None, :]
    return y.astype(np.float32)


# revision 52
# speedup vs baseline: 1.0103x; 1.0015x over previous
"""Local (banded) attention kernel for Trainium2, sharded over 8 NeuronCores.

Sharding: core c handles batch b=c//4 and heads 4*(c%4)..4*(c%4)+3.
Q/K projections and QK^T run as fp8 DoubleRow matmuls (host pre-quantizes
x and the scaled Q/K weight slices, permuted so each head's 64-dim split
lands as [32 partitions x 2 DR slots]).  The band mask is accumulated into
the energy PSUM via tiny fp8e5 DoubleRow matmuls (identity stationary x
precomputed -57344 panels), so exp() needs no separate mask pass.  V and
output projections run in f16; y partials stream out in f16 and the host
sums the 4 partials per batch in f32 and adds the output bias.
"""

import ml_dtypes
import numpy as np

import concourse.bass as bass
import concourse.mybir as mybir
from concourse import bacc
from concourse.tile import TileContext
from concourse.bass_utils import run_bass_kernel_spmd
from concourse.masks import make_identity

B, N, E, H, DH, WIN = 2, 2048, 1024, 16, 64, 128
HPC = 4              # heads per core
SL = HPC * DH        # feature slice per core (256)
NT = N // 128        # 16 query/key tiles
F32 = mybir.dt.float32
F16 = mybir.dt.float16
BF16 = mybir.dt.bfloat16
F8 = mybir.dt.float8e4
F8E5 = mybir.dt.float8e5
SCALE = 1.0 / 32.0   # 1/sqrt(E)
WSCALE = 32.0        # Q/K weights are shipped as fp8(32*W); undone in copies
MASKVAL = -57344.0   # exactly representable in e5m2; /32 => -1792 pre-exp
AUXW = 264           # aux rows: 0=bv, 1=ones, 2=zeros
KO = E // 128        # 8 contraction tiles
KP = KO // 2         # 4 DoubleRow contraction-pair tiles
DR = mybir.MatmulPerfMode.DoubleRow

_CACHED_NC = None


def _build_nc(nobias):
    nc = bacc.Bacc("TRN2", target_bir_lowering=False)

    x8_d = nc.dram_tensor("x8", [E, N], F8, kind="ExternalInput")
    x1_d = nc.dram_tensor("x1", [E, N], F8, kind="ExternalInput")
    wq8_d = nc.dram_tensor("wq8", [E, 2, 128], F8, kind="ExternalInput")
    wk8_d = nc.dram_tensor("wk8", [E, 2, 128], F8, kind="ExternalInput")
    wv0_d = nc.dram_tensor("wv0", [E, SL], F8, kind="ExternalInput")
    wv1_d = nc.dram_tensor("wv1", [E, SL], F8, kind="ExternalInput")
    wv0s_d = nc.dram_tensor("wv0s", [E, SL], F8, kind="ExternalInput")
    wp_d = nc.dram_tensor("wp", [SL, E], F16, kind="ExternalInput")
    bqk_d = nc.dram_tensor("bqk", [128, 6], F32, kind="ExternalInput")
    msk_d = nc.dram_tensor("msk", [128, 2, 256], F8E5, kind="ExternalInput")
    idz_d = nc.dram_tensor("idz", [128, 2, 128], F8E5, kind="ExternalInput")
    y_d = nc.dram_tensor("y", [N, E], F16, kind="ExternalOutput")

    with TileContext(nc) as tc:
        with (
            tc.tile_pool(name="const", bufs=1) as const,
            tc.tile_pool(name="persist", bufs=1) as persist,
            tc.tile_pool(name="io", bufs=3) as io,
            tc.tile_pool(name="small", bufs=6) as small,
            tc.tile_pool(name="att2p", bufs=3) as att2p,
            tc.tile_pool(name="strips", bufs=20) as strip_pool,
            tc.tile_pool(name="ps_mm", bufs=2, space="PSUM") as ps_mm,
            tc.tile_pool(name="ps_e", bufs=4, space="PSUM") as ps_e,
            tc.tile_pool(name="ps_ut", bufs=2, space="PSUM") as ps_ut,
        ):
            # ---- DMAs ordered by first use; the first few go out on the
            # idle Act/DVE queues so their DGE prologues overlap ----
            wq_sb = persist.tile([128, KP, 2, 2, 128], F8)
            wk_sb = persist.tile([128, KP, 2, 2, 128], F8)
            x8_sb = persist.tile([128, KO, N], F8)
            x1_sb = persist.tile([128, KO, N], F8)
            wv0_sb = persist.tile([128, KO, SL], F8)
            wv1_sb = persist.tile([128, KO, SL], F8)
            wv0s_sb = persist.tile([128, KO, SL], F8)
            wp_sb = persist.tile([128, 2, E], F16)
            x8_ap = x8_d.ap().rearrange("(ko p) n -> p ko n", p=128)
            x1_ap = x1_d.ap().rearrange("(ko p) n -> p ko n", p=128)
            NCH = 4
            CW = N // NCH  # 512

            sa, sb = slice(0, 256), slice(256, CW)
            nc.scalar.dma_start(
                wq_sb[:], wq8_d.ap().rearrange(
                    "(kp dr p) i m -> p kp dr i m", p=128, dr=2))
            nc.scalar.dma_start(
                wk_sb[:], wk8_d.ap().rearrange(
                    "(kp dr p) i m -> p kp dr i m", p=128, dr=2))
            bqk = const.tile([128, 6], F32)
            nc.scalar.dma_start(bqk[:], bqk_d.ap())
            msk = const.tile([128, 2, 256], F8E5)
            idz = const.tile([128, 2, 128], F8E5)
            nc.scalar.dma_start(msk[:], msk_d.ap())
            nc.scalar.dma_start(idz[:], idz_d.ap())
            nc.sync.dma_start(x8_sb[:, :, sa], x8_ap[:, :, sa])
            nc.sync.dma_start(x8_sb[:, :, sb], x8_ap[:, :, sb])
            wvr = "(ko p) m -> p ko m"
            nc.sync.dma_start(wv0_sb[:], wv0_d.ap().rearrange(wvr, p=128))
            nc.sync.dma_start(wv1_sb[:], wv1_d.ap().rearrange(wvr, p=128))
            nc.sync.dma_start(wv0s_sb[:], wv0s_d.ap().rearrange(wvr, p=128))
            s0 = slice(0, CW)
            nc.sync.dma_start(x1_sb[:, :, s0], x1_ap[:, :, s0])
            ident = const.tile([128, 128], BF16)
            make_identity(nc, ident[:])
            # warm the PE p-state while input DMAs are in flight: the clock
            # ramps to full after ~3us of continuous matmul activity
            warm = ps_mm.tile([128, 2, 256], F32, tag="mm", name="warm")
            for wi in range(40):
                nc.tensor.matmul(
                    warm[:, 0, :128], lhsT=ident[:], rhs=ident[:],
                    start=(wi == 0), stop=(wi == 39),
                    skip_group_check=True)
            nc.sync.dma_start(
                wp_sb[:], wp_d.ap().rearrange("(g p) f -> p g f", p=128))
            for c4 in range(1, NCH):
                s = slice(c4 * CW, (c4 + 1) * CW)
                nc.sync.dma_start(x8_sb[:, :, s], x8_ap[:, :, s])
                nc.sync.dma_start(x1_sb[:, :, s], x1_ap[:, :, s])

            # ---- projection outputs ----
            # q8/k8 layout (hw only allows AP base partitions 0/32/64):
            #   head 0: partitions  0-31, slots 0/1 = dh halves  (DoubleRow)
            #   head 1: partitions 32-63, slots 0/1 = dh halves  (DoubleRow)
            #   head 2: partitions 64-127, slot 0 = full dh      (plain fp8)
            #   head 3: partitions 64-127, slot 1 = full dh      (plain fp8)
            q8 = persist.tile([128, 2, N], F8, name="q8", tag="q8")
            k8 = persist.tile([128, 2, N], F8, name="k8", tag="k8")
            vaug = persist.tile([128, NT, HPC, DH + 1], BF16)
            nc.gpsimd.memset(vaug[:, :, :, DH], 1.0)

            # ---- phase 2: QKV emitted as drip-able units so projection
            # matmuls (PE) interleave with strip exp/copies (Act/DVE) ----
            def emit_qk_unit(ch, proj, c2):
                w_sb, out_t, bc = ((wq_sb, q8, 0), (wk_sb, k8, 2))[proj]
                cs = slice(ch * CW + c2 * 256, ch * CW + c2 * 256 + 256)
                ps = ps_mm.tile([128, 2, 256], F32, tag="mm", name="ps_qk")
                for i in range(2):
                    for kp in range(KP):
                        nc.tensor.matmul(
                            ps[:, i, :],
                            lhsT=w_sb[:, kp, :, i, :],
                            rhs=x8_sb[:, 2 * kp:2 * kp + 2, cs],
                            start=(kp == 0), stop=(kp == KP - 1),
                            perf_mode=DR)
                if nobias:
                    if proj == 0:
                        nc.scalar.activation(
                            out_t[:, :, cs], ps[:],
                            mybir.ActivationFunctionType.Identity,
                            scale=1.0 / WSCALE)
                    else:
                        nc.vector.tensor_scalar_mul(
                            out_t[:, :, cs], ps[:], 1.0 / WSCALE)
                else:
                    nc.scalar.activation(
                        out_t[:, 0, cs], ps[:, 0, :],
                        mybir.ActivationFunctionType.Identity,
                        scale=1.0 / WSCALE, bias=bqk[:, bc:bc + 1])
                    nc.vector.tensor_scalar(
                        out_t[:, 1, cs], ps[:, 1, :],
                        1.0 / WSCALE, bqk[:, bc + 1:bc + 2],
                        mybir.AluOpType.mult, mybir.AluOpType.add)

            def emit_v_unit(nt):
                # 32*v ~= X0@W0 + X0@(W1/64) + X1@(W0/64); /32 in the copy
                ps = ps_mm.tile([128, 2, 256], F32, tag="mm", name="ps_v")
                psv = ps[:, 0, :]
                rs = slice(nt * 128, (nt + 1) * 128)
                terms = ((x8_sb, wv0_sb), (x8_sb, wv1_sb), (x1_sb, wv0s_sb))
                for ti, (xs, ws) in enumerate(terms):
                    for kp in range(KP):
                        nc.tensor.matmul(
                            psv, lhsT=xs[:, 2 * kp:2 * kp + 2, rs],
                            rhs=ws[:, 2 * kp:2 * kp + 2, :],
                            start=(ti == 0 and kp == 0),
                            stop=(ti == 2 and kp == KP - 1),
                            perf_mode=DR)
                nc.vector.tensor_scalar_mul(
                    vaug[:, nt, :, :DH],
                    psv.rearrange("p (h d) -> p h d", d=DH), 1.0 / WSCALE)

            def chunk_units(ch, qk_only=False, v_only=False):
                # (sort_key, deadline_kj, unit): qk needed by strips at
                # kj=4ch-1, v unit nt by process_tile(nt-1) at kj=nt+1.
                # sort keys stagger v after qk (v DMAs arrive later).
                us = []
                if not v_only:
                    us += [(4 * ch - 1, 4 * ch - 1,
                            lambda p=p, c=c: emit_qk_unit(ch, p, c))
                           for p in range(2) for c in range(2)]
                if not qk_only:
                    us += [(n + 1, n + 1, lambda n=n: emit_v_unit(n))
                           for n in range(ch * NCH, (ch + 1) * NCH)]
                return us

            # ---- phase 3+4: banded attention, fused projection + store ----
            strips = {}

            def emit_strip_pair(hp, kj):
                """Strips for heads (2*hp, 2*hp+1) of key tile kj: QK^T +
                mask matmuls into a 2-bank psum tile, one fused exp."""
                lo, hi = max(0, kj - 1), min(NT - 1, kj + 1)
                w = (hi - lo + 1) * 128
                kjs = slice(kj * 128, (kj + 1) * 128)
                sps = slice(lo * 128, (hi + 1) * 128)
                panels = []
                if kj > 0:   # U panel sits where queries tile kj-1 lives
                    panels.append(((kj - 1 - lo) * 128, 0))
                if kj < NT - 1:  # L panel at queries tile kj+1
                    panels.append(((kj + 1 - lo) * 128, 128))
                for j in range(2):
                    h = 2 * hp + j
                    pe = ps_e.tile([128, 384], F32, tag="pe", name="pe")
                    if h < 2:
                        hs = slice(32 * h, 32 * h + 32)
                        nc.tensor.matmul(
                            pe[:, :w], lhsT=k8[hs, :, kjs],
                            rhs=q8[hs, :, sps],
                            start=True, stop=False, perf_mode=DR,
                            skip_group_check=True)
                    else:
                        i = h - 2
                        nc.tensor.matmul(
                            pe[:, :w], lhsT=k8[64:128, i, kjs],
                            rhs=q8[64:128, i, sps],
                            start=True, stop=False, skip_group_check=True)
                    for n_, (po, mo) in enumerate(panels):
                        nc.tensor.matmul(
                            pe[:, po:po + 128],
                            lhsT=idz[:],
                            rhs=msk[:, :, mo:mo + 128],
                            start=False, stop=(n_ == len(panels) - 1),
                            perf_mode=DR, skip_group_check=True)
                    st = strip_pool.tile(
                        [128, 384], BF16, tag="strip", name="st")
                    nc.scalar.activation(
                        st[:, :w], pe[:, :w],
                        mybir.ActivationFunctionType.Exp, scale=SCALE)
                    strips[(h, kj)] = (st, lo)

            tile_ao = {}

            def process_tile_a(t):
                ks = [k for k in (t - 1, t, t + 1) if 0 <= k < NT]
                pu = ps_ut.tile([128, HPC, DH + 1], F32, tag="ut", name="pu")
                # PE executes in emission order, so the first matmul's
                # start=True zeroes the whole 2KB bank region before the
                # other 11 accumulate into their sub-slices.
                for h in range(HPC):
                    for i, k2 in enumerate(ks):
                        st, lo2 = strips[(h, k2)]
                        col = (t - lo2) * 128
                        nc.tensor.matmul(
                            pu[:, h, :], lhsT=st[:, col:col + 128],
                            rhs=vaug[:, k2, h, :],
                            start=(h == 0 and i == 0),
                            stop=(h == HPC - 1 and i == len(ks) - 1),
                            skip_group_check=True)
                rec = small.tile([128, HPC], F32, tag="rec", name="rec")
                nc.vector.reciprocal(rec[:], pu[:, :, DH])
                ao = small.tile([128, HPC, DH], BF16, tag="ao", name="ao")
                nc.vector.tensor_tensor(
                    ao[:], pu[:, :, :DH],
                    rec[:].unsqueeze(2).broadcast_to([128, HPC, DH]),
                    mybir.AluOpType.mult)
                tile_ao[t] = ao

            def process_tile_b(t):
                ts_ = slice(t * 128, (t + 1) * 128)
                ao = tile_ao.pop(t)
                ptb = ps_ut.tile([128, 2, 128], BF16, tag="ut", name="ptb")
                for g in range(2):
                    nc.tensor.transpose(
                        ptb[:, g, :], ao[:, 2 * g:2 * g + 2, :], ident[:])
                att2 = att2p.tile([128, 2, 128], BF16, tag="att2", name="att2")
                if nobias:
                    nc.vector.tensor_copy(
                        att2.rearrange("p g n -> p (g n)"),
                        ptb.rearrange("p g n -> p (g n)"))
                else:
                    for g in range(2):
                        nc.vector.tensor_scalar_add(
                            att2[:, g, :], ptb[:, g, :], bqk[:, 4 + g:5 + g])
                # fused output projection for this token tile
                y_sb = io.tile([128, E], F16, tag="y", name="y_sb")
                for fc in range(2):
                    ps = ps_mm.tile([128, 2, 256], F32, tag="mm", name="ps_y")
                    psy = ps.rearrange("p a b -> p (a b)")
                    fs = slice(fc * 512, (fc + 1) * 512)
                    for g in range(2):
                        nc.tensor.matmul(
                            psy,
                            lhsT=att2[:, g, :],
                            rhs=wp_sb[:, g, fs],
                            start=(g == 0), stop=(g == 1))
                    if fc == 0 and t % 2 == 0:
                        nc.scalar.activation(
                            y_sb[:, fs], psy,
                            mybir.ActivationFunctionType.Copy)
                    else:
                        nc.vector.tensor_copy(y_sb[:, fs], psy)
                nc.sync.dma_start(y_d[ts_, :], y_sb[:])

            # interleave: chunk 0's qkv first, then per-kj strips/tile with
            # 2-3 qkv units of upcoming chunks dripped in between.  strip kj
            # needs q/k tokens up to 128*(kj+2) => all chunks < (kj+3)/4.
            LEAD = 3
            pending = []  # (sort_key, deadline_kj, unit)
            for ch in range(1, NCH):
                pending.extend(chunk_units(ch))
            pending.sort(key=lambda du: du[0])
            for _, _, u in chunk_units(0):
                u()
            for kj in range(NT):
                while pending and min(d for _, d, _ in pending) <= kj:
                    i = next(i for i, (_, d, _) in enumerate(pending)
                             if d <= kj)
                    pending.pop(i)[2]()
                for hp in range(2):
                    emit_strip_pair(hp, kj)
                if kj >= LEAD:
                    process_tile_a(kj - LEAD)
                    process_tile_b(kj - LEAD)
                # steady drip keeps PE fed between strips through the tail
                drip = 1 if len(pending) <= (NT - 1 - kj) else 2
                for _ in range(min(drip, len(pending))):
                    pending.pop(0)[2]()
            for t in range(NT - LEAD, NT):
                process_tile_a(t)
                process_tile_b(t)

    nc.compile()
    return nc


_CACHED_NOBIAS = None


def _get_nc(nobias):
    global _CACHED_NC, _CACHED_NOBIAS
    if _CACHED_NC is None or _CACHED_NOBIAS != nobias:
        _CACHED_NC = _build_nc(nobias)
        _CACHED_NOBIAS = nobias
    return _CACHED_NC


def _prep_core(x_b, Wq, bq, Wk, bk, Wv, bv, Wp, gq):
    f8 = ml_dtypes.float8_e4m3
    f8e5 = ml_dtypes.float8_e5m2
    sl = slice(SL * gq, SL * (gq + 1))
    xT = np.ascontiguousarray(x_b.T).astype(np.float32)

    # feature index (within this core's 256-slice) at (slot i, partition m):
    #   m<32: head0 dh=32i+m; 32<=m<64: head1 dh=32i+(m-32);
    #   m>=64: head (2+i), dh=m-64
    fidx = np.zeros((2, 128), np.int64)
    m = np.arange(128)
    for i in range(2):
        fidx[i, :32] = 32 * i + m[:32]
        fidx[i, 32:64] = 64 + 32 * i + (m[32:64] - 32)
        fidx[i, 64:] = 64 * (2 + i) + (m[64:] - 64)

    def qk_weight(W):
        w = np.ascontiguousarray(W[sl].T).astype(np.float32) * WSCALE
        return w[:, fidx].astype(f8)  # [E, 2, 128]

    def qk_bias(b):
        return np.asarray(b, np.float32)[sl][fidx]  # [2, 128]

    bq2, bk2 = qk_bias(bq), qk_bias(bk)
    bvs = np.asarray(bv, np.float32)[sl]
    bqk = np.stack([bq2[0], bq2[1], bk2[0], bk2[1],
                    bvs[:128], bvs[128:]], axis=1)  # [128, 6]

    # band-mask panels: U keeps qcol >= p, L keeps qcol <= p
    pi = np.arange(128)
    msk = np.zeros((128, 2, 256), np.float32)
    msk[:, 0, :128] = np.where(pi[None, :] >= pi[:, None], 0.0, MASKVAL)
    msk[:, 0, 128:] = np.where(pi[None, :] <= pi[:, None], 0.0, MASKVAL)
    idz = np.zeros((128, 2, 128), np.float32)
    idz[:, 0, :] = np.eye(128, dtype=np.float32)

    x0 = xT.astype(f8)
    x1 = ((xT - x0.astype(np.float32)) * 64.0).astype(f8)
    wvT = np.ascontiguousarray(np.asarray(Wv, np.float32)[sl].T)
    wv0 = (WSCALE * wvT).astype(f8)
    wv1 = ((WSCALE * wvT - wv0.astype(np.float32)) * 64.0).astype(f8)
    wv1 = (wv1.astype(np.float32) / 64.0).astype(f8)   # ~= rw, exact-ish
    wv0s = (wv0.astype(np.float32) / 64.0).astype(f8)  # = W0/64, exact-ish
    return {
        "x8": x0,
        "x1": x1,
        "wq8": qk_weight(Wq),
        "wk8": qk_weight(Wk),
        "wv0": wv0,
        "wv1": wv1,
        "wv0s": wv0s,
        "wp": np.ascontiguousarray(np.asarray(Wp, np.float32)[:, sl].T).astype(
            np.float16),
        "bqk": np.ascontiguousarray(bqk),
        "msk": msk.astype(f8e5),
        "idz": idz.astype(f8e5),
    }


def kernel(x, Wq, bq, Wk, bk, Wv, bv, Wp, bp):
    nobias = bool(
        not np.any(np.asarray(bq)) and not np.any(np.asarray(bk))
        and not np.any(np.asarray(bv)))
    nc = _get_nc(nobias)
    x = np.asarray(x, np.float32)
    in_maps = []
    for c in range(8):
        b, gq = c // 4, c % 4
        m = _prep_core(x[b], np.asarray(Wq, np.float32), bq,
                       np.asarray(Wk, np.float32), bk,
                       np.asarray(Wv, np.float32), bv,
                       np.asarray(Wp, np.float32), gq)
        in_maps.append(m)
    res = run_bass_kernel_spmd(nc, in_maps, core_ids=list(range(8)))
    ys = [res.results[c]["y"].astype(np.float32) for c in range(8)]
    bp = np.asarray(bp, np.float32)
    y = np.stack([
        ys[0] + ys[1] + ys[2] + ys[3],
        ys[4] + ys[5] + ys[6] + ys[7],
    ]) + bp[None, None, :]
    return y.astype(np.float32)


# revision 53
# speedup vs baseline: 1.0169x; 1.0066x over previous
"""Local (banded) attention kernel for Trainium2, sharded over 8 NeuronCores.

Sharding: core c handles batch b=c//4 and heads 4*(c%4)..4*(c%4)+3.
Q/K projections and QK^T run as fp8 DoubleRow matmuls (host pre-quantizes
x and the scaled Q/K weight slices, permuted so each head's 64-dim split
lands as [32 partitions x 2 DR slots]).  The band mask is accumulated into
the energy PSUM via tiny fp8e5 DoubleRow matmuls (identity stationary x
precomputed -57344 panels), so exp() needs no separate mask pass.  V and
output projections run in f16; y partials stream out in f16 and the host
sums the 4 partials per batch in f32 and adds the output bias.
"""

import ml_dtypes
import numpy as np

import concourse.bass as bass
import concourse.mybir as mybir
from concourse import bacc
from concourse.tile import TileContext
from concourse.bass_utils import run_bass_kernel_spmd
from concourse.masks import make_identity

B, N, E, H, DH, WIN = 2, 2048, 1024, 16, 64, 128
HPC = 4              # heads per core
SL = HPC * DH        # feature slice per core (256)
NT = N // 128        # 16 query/key tiles
F32 = mybir.dt.float32
F16 = mybir.dt.float16
BF16 = mybir.dt.bfloat16
F8 = mybir.dt.float8e4
F8E5 = mybir.dt.float8e5
SCALE = 1.0 / 32.0   # 1/sqrt(E)
WSCALE = 32.0        # Q/K weights are shipped as fp8(32*W); undone in copies
MASKVAL = -57344.0   # exactly representable in e5m2; /32 => -1792 pre-exp
AUXW = 264           # aux rows: 0=bv, 1=ones, 2=zeros
KO = E // 128        # 8 contraction tiles
KP = KO // 2         # 4 DoubleRow contraction-pair tiles
DR = mybir.MatmulPerfMode.DoubleRow

_CACHED_NC = None


def _build_nc(nobias):
    nc = bacc.Bacc("TRN2", target_bir_lowering=False)

    x8_d = nc.dram_tensor("x8", [E, N], F8, kind="ExternalInput")
    x1_d = nc.dram_tensor("x1", [E, N], F8, kind="ExternalInput")
    wq8_d = nc.dram_tensor("wq8", [E, 2, 128], F8, kind="ExternalInput")
    wk8_d = nc.dram_tensor("wk8", [E, 2, 128], F8, kind="ExternalInput")
    wv0_d = nc.dram_tensor("wv0", [E, SL], F8, kind="ExternalInput")
    wv1_d = nc.dram_tensor("wv1", [E, SL], F8, kind="ExternalInput")
    wv0s_d = nc.dram_tensor("wv0s", [E, SL], F8, kind="ExternalInput")
    wp_d = nc.dram_tensor("wp", [SL, E], F16, kind="ExternalInput")
    bqk_d = nc.dram_tensor("bqk", [128, 6], F32, kind="ExternalInput")
    msk_d = nc.dram_tensor("msk", [128, 2, 256], F8E5, kind="ExternalInput")
    idz_d = nc.dram_tensor("idz", [128, 2, 128], F8E5, kind="ExternalInput")
    y_d = nc.dram_tensor("y", [N, E], F16, kind="ExternalOutput")

    with TileContext(nc) as tc:
        with (
            tc.tile_pool(name="const", bufs=1) as const,
            tc.tile_pool(name="persist", bufs=1) as persist,
            tc.tile_pool(name="io", bufs=3) as io,
            tc.tile_pool(name="small", bufs=6) as small,
            tc.tile_pool(name="att2p", bufs=3) as att2p,
            tc.tile_pool(name="strips", bufs=20) as strip_pool,
            tc.tile_pool(name="ps_mm", bufs=2, space="PSUM") as ps_mm,
            tc.tile_pool(name="ps_e", bufs=4, space="PSUM") as ps_e,
            tc.tile_pool(name="ps_ut", bufs=2, space="PSUM") as ps_ut,
        ):
            # ---- DMAs ordered by first use; the first few go out on the
            # idle Act/DVE queues so their DGE prologues overlap ----
            wq_sb = persist.tile([128, KP, 2, 2, 128], F8)
            wk_sb = persist.tile([128, KP, 2, 2, 128], F8)
            x8_sb = persist.tile([128, KO, N], F8)
            x1_sb = persist.tile([128, KO, N], F8)
            wv0_sb = persist.tile([128, KO, SL], F8)
            wv1_sb = persist.tile([128, KO, SL], F8)
            wv0s_sb = persist.tile([128, KO, SL], F8)
            wp_sb = persist.tile([128, 2, E], F16)
            x8_ap = x8_d.ap().rearrange("(ko p) n -> p ko n", p=128)
            x1_ap = x1_d.ap().rearrange("(ko p) n -> p ko n", p=128)
            NCH = 4
            CW = N // NCH  # 512

            sa, sb = slice(0, 256), slice(256, CW)
            nc.scalar.dma_start(
                wq_sb[:], wq8_d.ap().rearrange(
                    "(kp dr p) i m -> p kp dr i m", p=128, dr=2))
            nc.scalar.dma_start(
                wk_sb[:], wk8_d.ap().rearrange(
                    "(kp dr p) i m -> p kp dr i m", p=128, dr=2))
            bqk = const.tile([128, 6], F32)
            nc.scalar.dma_start(bqk[:], bqk_d.ap())
            msk = const.tile([128, 2, 256], F8E5)
            idz = const.tile([128, 2, 128], F8E5)
            nc.scalar.dma_start(msk[:], msk_d.ap())
            nc.scalar.dma_start(idz[:], idz_d.ap())
            nc.sync.dma_start(x8_sb[:, :, sa], x8_ap[:, :, sa])
            nc.sync.dma_start(x8_sb[:, :, sb], x8_ap[:, :, sb])
            wvr = "(ko p) m -> p ko m"
            nc.sync.dma_start(wv0_sb[:], wv0_d.ap().rearrange(wvr, p=128))
            nc.sync.dma_start(wv1_sb[:], wv1_d.ap().rearrange(wvr, p=128))
            nc.sync.dma_start(wv0s_sb[:], wv0s_d.ap().rearrange(wvr, p=128))
            s0 = slice(0, CW)
            nc.sync.dma_start(x1_sb[:, :, s0], x1_ap[:, :, s0])
            ident = const.tile([128, 128], BF16)
            make_identity(nc, ident[:])
            # warm the PE p-state while input DMAs are in flight: the clock
            # ramps to full after ~3us of continuous matmul activity
            warm = ps_mm.tile([128, 2, 256], F32, tag="mm", name="warm")
            for wi in range(40):
                nc.tensor.matmul(
                    warm[:, 0, :128], lhsT=ident[:], rhs=ident[:],
                    start=(wi == 0), stop=(wi == 39),
                    skip_group_check=True)
            nc.sync.dma_start(
                wp_sb[:], wp_d.ap().rearrange("(g p) f -> p g f", p=128))
            for c4 in range(1, NCH):
                s = slice(c4 * CW, (c4 + 1) * CW)
                nc.sync.dma_start(x8_sb[:, :, s], x8_ap[:, :, s])
                nc.sync.dma_start(x1_sb[:, :, s], x1_ap[:, :, s])

            # ---- projection outputs ----
            # q8/k8 layout (hw only allows AP base partitions 0/32/64):
            #   head 0: partitions  0-31, slots 0/1 = dh halves  (DoubleRow)
            #   head 1: partitions 32-63, slots 0/1 = dh halves  (DoubleRow)
            #   head 2: partitions 64-127, slot 0 = full dh      (plain fp8)
            #   head 3: partitions 64-127, slot 1 = full dh      (plain fp8)
            q8 = persist.tile([128, 2, N], F8, name="q8", tag="q8")
            k8 = persist.tile([128, 2, N], F8, name="k8", tag="k8")
            vaug = persist.tile([128, NT, HPC, DH + 1], BF16)
            nc.gpsimd.memset(vaug[:, :, :, DH], 1.0)

            # ---- phase 2: QKV emitted as drip-able units so projection
            # matmuls (PE) interleave with strip exp/copies (Act/DVE) ----
            def emit_qk_unit(ch, proj, c2):
                w_sb, out_t, bc = ((wq_sb, q8, 0), (wk_sb, k8, 2))[proj]
                cs = slice(ch * CW + c2 * 256, ch * CW + c2 * 256 + 256)
                ps = ps_mm.tile([128, 2, 256], F32, tag="mm", name="ps_qk")
                for i in range(2):
                    for kp in range(KP):
                        nc.tensor.matmul(
                            ps[:, i, :],
                            lhsT=w_sb[:, kp, :, i, :],
                            rhs=x8_sb[:, 2 * kp:2 * kp + 2, cs],
                            start=(kp == 0), stop=(kp == KP - 1),
                            perf_mode=DR)
                if nobias:
                    if proj == 0:
                        nc.scalar.activation(
                            out_t[:, :, cs], ps[:],
                            mybir.ActivationFunctionType.Identity,
                            scale=1.0 / WSCALE)
                    else:
                        nc.vector.tensor_scalar_mul(
                            out_t[:, :, cs], ps[:], 1.0 / WSCALE)
                else:
                    nc.scalar.activation(
                        out_t[:, 0, cs], ps[:, 0, :],
                        mybir.ActivationFunctionType.Identity,
                        scale=1.0 / WSCALE, bias=bqk[:, bc:bc + 1])
                    nc.vector.tensor_scalar(
                        out_t[:, 1, cs], ps[:, 1, :],
                        1.0 / WSCALE, bqk[:, bc + 1:bc + 2],
                        mybir.AluOpType.mult, mybir.AluOpType.add)

            def emit_v_unit(nt):
                # 32*v ~= X0@W0 + X0@(W1/64) + X1@(W0/64); /32 in the copy
                ps = ps_mm.tile([128, 2, 256], F32, tag="mm", name="ps_v")
                psv = ps[:, 0, :]
                rs = slice(nt * 128, (nt + 1) * 128)
                terms = ((x8_sb, wv0_sb), (x8_sb, wv1_sb), (x1_sb, wv0s_sb))
                for ti, (xs, ws) in enumerate(terms):
                    for kp in range(KP):
                        nc.tensor.matmul(
                            psv, lhsT=xs[:, 2 * kp:2 * kp + 2, rs],
                            rhs=ws[:, 2 * kp:2 * kp + 2, :],
                            start=(ti == 0 and kp == 0),
                            stop=(ti == 2 and kp == KP - 1),
                            perf_mode=DR)
                nc.vector.tensor_scalar_mul(
                    vaug[:, nt, :, :DH],
                    psv.rearrange("p (h d) -> p h d", d=DH), 1.0 / WSCALE)

            def chunk_units(ch, qk_only=False, v_only=False):
                # (sort_key, deadline_kj, unit): qk needed by strips at
                # kj=4ch-1, v unit nt by process_tile(nt-1) at kj=nt+1.
                # sort keys stagger v after qk (v DMAs arrive later).
                us = []
                if not v_only:
                    # chunk 0 c2=1 units can lag: strips kj=1 are the first
                    # to read tokens 256-511
                    us += [(4 * ch - 1 if ch else c, 4 * ch - 1 if ch else c,
                            lambda p=p, c=c: emit_qk_unit(ch, p, c))
                           for c in range(2) for p in range(2)]
                if not qk_only:
                    us += [(n + 1, n + 1, lambda n=n: emit_v_unit(n))
                           for n in range(ch * NCH, (ch + 1) * NCH)]
                return us

            # ---- phase 3+4: banded attention, fused projection + store ----
            strips = {}

            def emit_strip_pair(hp, kj):
                """Strips for heads (2*hp, 2*hp+1) of key tile kj: QK^T +
                mask matmuls into a 2-bank psum tile, one fused exp."""
                lo, hi = max(0, kj - 1), min(NT - 1, kj + 1)
                w = (hi - lo + 1) * 128
                kjs = slice(kj * 128, (kj + 1) * 128)
                sps = slice(lo * 128, (hi + 1) * 128)
                panels = []
                if kj > 0:   # U panel sits where queries tile kj-1 lives
                    panels.append(((kj - 1 - lo) * 128, 0))
                if kj < NT - 1:  # L panel at queries tile kj+1
                    panels.append(((kj + 1 - lo) * 128, 128))
                for j in range(2):
                    h = 2 * hp + j
                    pe = ps_e.tile([128, 384], F32, tag="pe", name="pe")
                    if h < 2:
                        hs = slice(32 * h, 32 * h + 32)
                        nc.tensor.matmul(
                            pe[:, :w], lhsT=k8[hs, :, kjs],
                            rhs=q8[hs, :, sps],
                            start=True, stop=False, perf_mode=DR,
                            skip_group_check=True)
                    else:
                        i = h - 2
                        nc.tensor.matmul(
                            pe[:, :w], lhsT=k8[64:128, i, kjs],
                            rhs=q8[64:128, i, sps],
                            start=True, stop=False, skip_group_check=True)
                    for n_, (po, mo) in enumerate(panels):
                        nc.tensor.matmul(
                            pe[:, po:po + 128],
                            lhsT=idz[:],
                            rhs=msk[:, :, mo:mo + 128],
                            start=False, stop=(n_ == len(panels) - 1),
                            perf_mode=DR, skip_group_check=True)
                    st = strip_pool.tile(
                        [128, 384], BF16, tag="strip", name="st")
                    nc.scalar.activation(
                        st[:, :w], pe[:, :w],
                        mybir.ActivationFunctionType.Exp, scale=SCALE)
                    strips[(h, kj)] = (st, lo)

            tile_ao = {}

            def process_tile_a(t):
                ks = [k for k in (t - 1, t, t + 1) if 0 <= k < NT]
                pu = ps_ut.tile([128, HPC, DH + 1], F32, tag="ut", name="pu")
                # PE executes in emission order, so the first matmul's
                # start=True zeroes the whole 2KB bank region before the
                # other 11 accumulate into their sub-slices.
                for h in range(HPC):
                    for i, k2 in enumerate(ks):
                        st, lo2 = strips[(h, k2)]
                        col = (t - lo2) * 128
                        nc.tensor.matmul(
                            pu[:, h, :], lhsT=st[:, col:col + 128],
                            rhs=vaug[:, k2, h, :],
                            start=(h == 0 and i == 0),
                            stop=(h == HPC - 1 and i == len(ks) - 1),
                            skip_group_check=True)
                rec = small.tile([128, HPC], F32, tag="rec", name="rec")
                nc.vector.reciprocal(rec[:], pu[:, :, DH])
                ao = small.tile([128, HPC, DH], BF16, tag="ao", name="ao")
                nc.vector.tensor_tensor(
                    ao[:], pu[:, :, :DH],
                    rec[:].unsqueeze(2).broadcast_to([128, HPC, DH]),
                    mybir.AluOpType.mult)
                tile_ao[t] = ao

            def process_tile_b(t):
                ts_ = slice(t * 128, (t + 1) * 128)
                ao = tile_ao.pop(t)
                ptb = ps_ut.tile([128, 2, 128], BF16, tag="ut", name="ptb")
                for g in range(2):
                    nc.tensor.transpose(
                        ptb[:, g, :], ao[:, 2 * g:2 * g + 2, :], ident[:])
                att2 = att2p.tile([128, 2, 128], BF16, tag="att2", name="att2")
                if nobias:
                    nc.vector.tensor_copy(
                        att2.rearrange("p g n -> p (g n)"),
                        ptb.rearrange("p g n -> p (g n)"))
                else:
                    for g in range(2):
                        nc.vector.tensor_scalar_add(
                            att2[:, g, :], ptb[:, g, :], bqk[:, 4 + g:5 + g])
                # fused output projection for this token tile
                y_sb = io.tile([128, E], F16, tag="y", name="y_sb")
                for fc in range(2):
                    ps = ps_mm.tile([128, 2, 256], F32, tag="mm", name="ps_y")
                    psy = ps.rearrange("p a b -> p (a b)")
                    fs = slice(fc * 512, (fc + 1) * 512)
                    for g in range(2):
                        nc.tensor.matmul(
                            psy,
                            lhsT=att2[:, g, :],
                            rhs=wp_sb[:, g, fs],
                            start=(g == 0), stop=(g == 1))
                    if fc == 0 and t % 2 == 0:
                        nc.scalar.activation(
                            y_sb[:, fs], psy,
                            mybir.ActivationFunctionType.Copy)
                    else:
                        nc.vector.tensor_copy(y_sb[:, fs], psy)
                nc.sync.dma_start(y_d[ts_, :], y_sb[:])

            # interleave: chunk 0's qkv first, then per-kj strips/tile with
            # 2-3 qkv units of upcoming chunks dripped in between.  strip kj
            # needs q/k tokens up to 128*(kj+2) => all chunks < (kj+3)/4.
            LEAD = 3
            pending = []  # (sort_key, deadline_kj, unit)
            for ch in range(1, NCH):
                pending.extend(chunk_units(ch))
            pending.sort(key=lambda du: du[0])
            for _, _, u in chunk_units(0):
                u()
            for kj in range(NT):
                while pending and min(d for _, d, _ in pending) <= kj:
                    i = next(i for i, (_, d, _) in enumerate(pending)
                             if d <= kj)
                    pending.pop(i)[2]()
                for hp in range(2):
                    emit_strip_pair(hp, kj)
                if kj >= LEAD:
                    process_tile_a(kj - LEAD)
                    process_tile_b(kj - LEAD)
                # steady drip keeps PE fed between strips through the tail
                drip = 1 if len(pending) <= (NT - 1 - kj) else 2
                for _ in range(min(drip, len(pending))):
                    pending.pop(0)[2]()
            for t in range(NT - LEAD, NT):
                process_tile_a(t)
                process_tile_b(t)

    nc.compile()
    return nc


_CACHED_NOBIAS = None


def _get_nc(nobias):
    global _CACHED_NC, _CACHED_NOBIAS
    if _CACHED_NC is None or _CACHED_NOBIAS != nobias:
        _CACHED_NC = _build_nc(nobias)
        _CACHED_NOBIAS = nobias
    return _CACHED_NC


def _prep_core(x_b, Wq, bq, Wk, bk, Wv, bv, Wp, gq):
    f8 = ml_dtypes.float8_e4m3
    f8e5 = ml_dtypes.float8_e5m2
    sl = slice(SL * gq, SL * (gq + 1))
    xT = np.ascontiguousarray(x_b.T).astype(np.float32)

    # feature index (within this core's 256-slice) at (slot i, partition m):
    #   m<32: head0 dh=32i+m; 32<=m<64: head1 dh=32i+(m-32);
    #   m>=64: head (2+i), dh=m-64
    fidx = np.zeros((2, 128), np.int64)
    m = np.arange(128)
    for i in range(2):
        fidx[i, :32] = 32 * i + m[:32]
        fidx[i, 32:64] = 64 + 32 * i + (m[32:64] - 32)
        fidx[i, 64:] = 64 * (2 + i) + (m[64:] - 64)

    def qk_weight(W):
        w = np.ascontiguousarray(W[sl].T).astype(np.float32) * WSCALE
        return w[:, fidx].astype(f8)  # [E, 2, 128]

    def qk_bias(b):
        return np.asarray(b, np.float32)[sl][fidx]  # [2, 128]

    bq2, bk2 = qk_bias(bq), qk_bias(bk)
    bvs = np.asarray(bv, np.float32)[sl]
    bqk = np.stack([bq2[0], bq2[1], bk2[0], bk2[1],
                    bvs[:128], bvs[128:]], axis=1)  # [128, 6]

    # band-mask panels: U keeps qcol >= p, L keeps qcol <= p
    pi = np.arange(128)
    msk = np.zeros((128, 2, 256), np.float32)
    msk[:, 0, :128] = np.where(pi[None, :] >= pi[:, None], 0.0, MASKVAL)
    msk[:, 0, 128:] = np.where(pi[None, :] <= pi[:, None], 0.0, MASKVAL)
    idz = np.zeros((128, 2, 128), np.float32)
    idz[:, 0, :] = np.eye(128, dtype=np.float32)

    x0 = xT.astype(f8)
    x1 = ((xT - x0.astype(np.float32)) * 64.0).astype(f8)
    wvT = np.ascontiguousarray(np.asarray(Wv, np.float32)[sl].T)
    wv0 = (WSCALE * wvT).astype(f8)
    wv1 = ((WSCALE * wvT - wv0.astype(np.float32)) * 64.0).astype(f8)
    wv1 = (wv1.astype(np.float32) / 64.0).astype(f8)   # ~= rw, exact-ish
    wv0s = (wv0.astype(np.float32) / 64.0).astype(f8)  # = W0/64, exact-ish
    return {
        "x8": x0,
        "x1": x1,
        "wq8": qk_weight(Wq),
        "wk8": qk_weight(Wk),
        "wv0": wv0,
        "wv1": wv1,
        "wv0s": wv0s,
        "wp": np.ascontiguousarray(np.asarray(Wp, np.float32)[:, sl].T).astype(
            np.float16),
        "bqk": np.ascontiguousarray(bqk),
        "msk": msk.astype(f8e5),
        "idz": idz.astype(f8e5),
    }


def kernel(x, Wq, bq, Wk, bk, Wv, bv, Wp, bp):
    nobias = bool(
        not np.any(np.asarray(bq)) and not np.any(np.asarray(bk))
        and not np.any(np.asarray(bv)))
    nc = _get_nc(nobias)
    x = np.asarray(x, np.float32)
    in_maps = []
    for c in range(8):
        b, gq = c // 4, c % 4
        m = _prep_core(x[b], np.asarray(Wq, np.float32), bq,
                       np.asarray(Wk, np.float32), bk,
                       np.asarray(Wv, np.float32), bv,
                       np.asarray(Wp, np.float32), gq)
        in_maps.append(m)
    res = run_bass_kernel_spmd(nc, in_maps, core_ids=list(range(8)))
    ys = [res.results[c]["y"].astype(np.float32) for c in range(8)]
    bp = np.asarray(bp, np.float32)
    y = np.stack([
        ys[0] + ys[1] + ys[2] + ys[3],
        ys[4] + ys[5] + ys[6] + ys[7],
    ]) + bp[None, None, :]
    return y.astype(np.float32)
